# revision 6
# baseline (speedup 1.0000x reference)
"""Trainium2 Bass kernel for nn_MultiHeadAttention_37538014167348.

The reference einsum is 'bhqk,bhvd->bhqd' (k and v are independent), so the
attention output factorizes into (sum_k softmax_weights) * (sum_v V). Softmax
rows sum to exactly 1 (also true for the complex softmax), hence:

    out[b, q, :] = (sum_s x[b, s, :]) @ Wv + S * bv     (independent of q)

Q/K/mask/softmax drop out entirely. The kernel computes the row-sum of x and a
complex [1,768]x[768,768] matvec; the host broadcasts the resulting row over
the 1024 sequence positions.

Sharding over 8 cores: (batch b in 0..3) x (contraction/feature half). Core
(b, j) reads x[b, :, j*384:(j+1)*384] (all 1024 rows, half the features,
3.15MB) and Wv[j*384:(j+1)*384, :] (half the weight rows, full 768 output
columns, 1.18MB bf16), and produces the partial matvec y_bj = u_bj @ Wv[half].
The host sums the two partials per batch and adds S*bv: no cross-core
communication, and per-core DMA drops from 7.44MB (previous version) to
4.33MB, which is what bounds the kernel (HBM ~358GB/s).

Pipeline (per core): x arrives as 2 column slabs (f32 cols [0:512) and
[512:768) of this core's half), each split into 4 row sub-DMAs so stage-1
matmuls chase the DMA tail. Stage 1 reduces rows with a stationary ones
[128,2] f32r matmul into psum chains A [2,512] / B [2,256] (f32r moving with
free >= 256 runs the PE at 1 cycle/row). Per 128-complex-feature chunk: DVE
deinterleaves re/im psum columns into rows, one PE transpose yields u columns
[a|b], DVE packs bf16 [a,b] and [-b,a] pairs, and 2+2 bf16 matmuls per chunk
accumulate y into psum [2,512]+[2,256] against the chunk-major weight planes
[C_cc | D_cc]. Stage 2 for chunks 0/1 overlaps the slab-2 DMA; only chunk 2's
tail (~3us) trails the last x byte.
"""

import os
import sys

import numpy as np

for _p in ("/opt/trn_rl_repo", "/root/.axon_site/_ro/trn_rl_repo"):
    if os.path.isdir(_p) and _p not in sys.path:
        sys.path.append(_p)

import ml_dtypes

from concourse import bacc, mybir
from concourse.tile import TileContext
from concourse.bass_utils import run_bass_kernel_spmd

B, S, H = 4, 1024, 768
HALF = H // 2           # complex features per core (384) = contraction half
NCORES = 8
P = 128                 # SBUF partitions
CC = HALF // P          # 3 contraction chunks of 128 complex features
F32 = mybir.dt.float32
F32R = mybir.dt.float32r
BF16 = mybir.dt.bfloat16

_NC = None
LAST_RESULTS = None     # stashed BassKernelResults for profiling in test.py


def _build():
    nc = bacc.Bacc(None, target_bir_lowering=False)

    # per-core x half: f32 view of x[b, :, j*384:(j+1)*384], contiguous
    x = nc.dram_tensor("x", [S, 2 * HALF], F32R, kind="ExternalInput")
    # per-chunk weight planes: wc{cc}[p, :] = bf16(Re(Wv)[joff+cc*128+p, :]),
    # wd{cc} likewise for Im. Split per-plane so each 196KB transfer lands
    # early on a fast queue and stage-2 never stalls on weights.
    wcs = [nc.dram_tensor(f"wc{cc}", [P, H], BF16, kind="ExternalInput")
           for cc in range(CC)]
    wds = [nc.dram_tensor(f"wd{cc}", [P, H], BF16, kind="ExternalInput")
           for cc in range(CC)]
    idf = nc.dram_tensor("idf", [2, 2], F32, kind="ExternalInput")
    onew = nc.dram_tensor("onew", [P, 2], F32R, kind="ExternalInput")
    o = nc.dram_tensor("o", [2, H], F32, kind="ExternalOutput")

    with TileContext(nc) as tc:
        with tc.tile_pool(name="sbuf", bufs=1) as pool, \
             tc.tile_pool(name="psum", bufs=1, space="PSUM") as psum:

            onesP = pool.tile([P, 2], F32R)
            id2 = pool.tile([2, 2], F32)
            wc_sb = [pool.tile([P, H], BF16, name=f"wcsb{cc}") for cc in range(CC)]
            wd_sb = [pool.tile([P, H], BF16, name=f"wdsb{cc}") for cc in range(CC)]

            # column-slab tiles, one per 128-complex-feature chunk: slab cc =
            # x[:, 256cc:256cc+256) as [128, 8, 256]; row sub s fills groups
            # 2s,2s+1 (partition p <- row 256s + 2p + g%2; sum-invariant).
            ts = [pool.tile([P, 8, 256], F32R, name=f"t{cc}") for cc in range(CC)]

            # ---- DMA triggers: per-queue programs in desired arrival order.
            # Queues sustain ~122GB/s (sync/scalar HW) and ~150GB/s but with a
            # ~2.5us late start (gpsimd swdge); shares are byte-balanced so
            # all three drain together (~1.44MB each).
            nc.sync.dma_start(out=onesP[:], in_=onew[:, :])
            nc.sync.dma_start(out=id2[:], in_=idf[:, :])

            def xsub(cc, s, eng):
                eng.dma_start(out=ts[cc][:, 2 * s:2 * s + 2, :],
                              in_=x[256 * s:256 * (s + 1),
                                    256 * cc:256 * cc + 256])

            xsub(0, 0, nc.scalar)
            xsub(0, 1, nc.gpsimd)
            xsub(0, 2, nc.sync)
            nc.scalar.dma_start(out=wc_sb[0][:], in_=wcs[0][:, :])
            nc.sync.dma_start(out=wd_sb[0][:], in_=wds[0][:, :])
            nc.gpsimd.dma_start(out=wc_sb[1][:], in_=wcs[1][:, :])
            xsub(0, 3, nc.scalar)
            nc.gpsimd.dma_start(out=wd_sb[1][:], in_=wds[1][:, :])
            xsub(1, 0, nc.sync)
            xsub(1, 1, nc.scalar)
            xsub(1, 2, nc.gpsimd)
            xsub(1, 3, nc.sync)
            xsub(2, 0, nc.gpsimd)
            xsub(2, 1, nc.sync)
            xsub(2, 2, nc.scalar)
            nc.scalar.dma_start(out=wc_sb[2][:], in_=wcs[2][:, :])
            nc.sync.dma_start(out=wd_sb[2][:], in_=wds[2][:, :])
            xsub(2, 3, nc.gpsimd)

            # ---- per-chunk pipeline: stage-1 chain -> deint -> transpose ->
            # bf16 pack -> stage-2 accumulate into oA/oB.
            u_ps = [psum.tile([2, 256], F32, name=f"u{cc}") for cc in range(CC)]
            u_row = [pool.tile([2, P], F32, name=f"urow{cc}") for cc in range(CC)]
            tp = [psum.tile([P, 2], F32, name=f"tp{cc}") for cc in range(CC)]
            u_ab = [pool.tile([P, 2], BF16, name=f"uab{cc}") for cc in range(CC)]
            u_bna = [pool.tile([P, 2], BF16, name=f"ubna{cc}") for cc in range(CC)]
            oA = psum.tile([2, 512], F32)
            oB = psum.tile([2, 256], F32)

            def deint(cc):
                # u_row[cc]: row0 = a (Re), row1 = b (Im), feats cc*128..+127
                v = u_ps[cc].rearrange("q (f two) -> q two f", two=2)
                nc.vector.tensor_copy(u_row[cc][0:2, :], v[0:2, 1, :])
                nc.vector.tensor_copy(u_row[cc][0:1, :], v[0:1, 0, :])

            def pack(cc):
                nc.tensor.transpose(tp[cc][:, :], u_row[cc][0:2, :], id2[:])
                nc.vector.tensor_copy(u_ab[cc][:], tp[cc][:])
                nc.vector.tensor_scalar_mul(u_bna[cc][:, 0:1], tp[cc][:, 1:2], -1.0)
                nc.vector.tensor_copy(u_bna[cc][:, 1:2], tp[cc][:, 0:1])

            def stage2(cc):
                nc.tensor.matmul(oA[:, :], u_ab[cc][:, :], wc_sb[cc][:, 0:512],
                                 start=(cc == 0), stop=False)
                nc.tensor.matmul(oB[:, :], u_ab[cc][:, :], wc_sb[cc][:, 512:768],
                                 start=(cc == 0), stop=False)
                nc.tensor.matmul(oA[:, :], u_bna[cc][:, :], wd_sb[cc][:, 0:512],
                                 start=False, stop=(cc == CC - 1))
                nc.tensor.matmul(oB[:, :], u_bna[cc][:, :], wd_sb[cc][:, 512:768],
                                 start=False, stop=(cc == CC - 1))

            for cc in range(CC):
                for g in range(8):
                    nc.tensor.matmul(u_ps[cc][:, :], onesP[:, 0:2],
                                     ts[cc][:, g, :],
                                     start=(g == 0), stop=(g == 7))
                deint(cc)
                pack(cc)
                stage2(cc)

            o_sb = pool.tile([2, H], F32)
            nc.vector.tensor_copy(o_sb[:, 0:512], oA[:])
            nc.vector.tensor_copy(o_sb[:, 512:768], oB[:])
            nc.scalar.dma_start(out=o[:, :], in_=o_sb[:])

    nc.finalize()
    return nc


def _get_nc():
    global _NC
    if _NC is None:
        _NC = _build()
    return _NC


def _pack_w(Wv, j):
    # per-chunk planes [128, H]: wc = Re rows, wd = Im rows (bf16)
    out = []
    for cc in range(CC):
        rows = slice(j * HALF + cc * P, j * HALF + (cc + 1) * P)
        wc = np.ascontiguousarray(Wv.real[rows, :].astype(ml_dtypes.bfloat16))
        wd = np.ascontiguousarray(Wv.imag[rows, :].astype(ml_dtypes.bfloat16))
        out.append((wc, wd))
    return out


def make_in_maps(x, Wv, bv):
    xf = np.ascontiguousarray(x).view(np.float32).reshape(B, S, 2 * H)
    Wv = np.ascontiguousarray(Wv)
    idv = np.eye(2, dtype=np.float32)
    wmaps = [_pack_w(Wv, j) for j in range(2)]
    in_maps = []
    for core in range(NCORES):
        b, j = divmod(core, 2)
        xc = np.ascontiguousarray(xf[b][:, j * 2 * HALF:(j + 1) * 2 * HALF])
        im = {"x": xc, "idf": idv,
              "onew": np.ones((P, 2), dtype=np.float32)}
        for cc in range(CC):
            im[f"wc{cc}"] = wmaps[j][cc][0]
            im[f"wd{cc}"] = wmaps[j][cc][1]
        in_maps.append(im)
    return in_maps


def kernel(x, Wq, bq, Wk, bk, Wv, bv, mask, trace=False):
    global LAST_RESULTS
    x = np.asarray(x)
    Wv = np.asarray(Wv)
    bv = np.asarray(bv)
    in_maps = make_in_maps(x, Wv, bv)
    res = run_bass_kernel_spmd(_get_nc(), in_maps, core_ids=list(range(NCORES)),
                               trace=trace)
    LAST_RESULTS = res
    sbv = (np.complex64(S) * bv).astype(np.complex64)
    row = np.empty((B, H), dtype=np.complex64)
    for b in range(B):
        o0 = res.results[2 * b]["o"]        # [2, 768] f32: partial j=0
        o1 = res.results[2 * b + 1]["o"]    # partial j=1
        row[b] = (o0[0] + o1[0]) + 1j * (o0[1] + o1[1])
    row += sbv[None, :]
    return np.ascontiguousarray(
        np.broadcast_to(row[:, None, :], (B, S, H)).astype(np.complex64))
